# revision 2
# baseline (speedup 1.0000x reference)
"""FP8ScaledLayer kernel for Trainium2 (8 NeuronCores, SPMD data-parallel).

Computes out = x @ (weight * scale[:, None]).T + bias with
  x: [4, 4096, 4096] fp32, weight: [4096, 4096] fp16,
  scale_weight: [4096] fp32, bias: [4096] fp32  ->  out [4, 4096, 4096] fp32.

Sharding: data-parallel over tokens (B*S = 16384 -> 2048 rows/core).
Weight is small (33.5 MB fp16) and replicated; x is large (268 MB) and
sharded, which keeps every core compute-bound instead of DMA-bound.

Per-core kernel:
  - cast x fp32 -> fp16 with a DRAM->DRAM SWDGE cast-DMA,
  - DMA-transpose (XBAR) x16 and weight into K-major SBUF tiles,
  - 16x8x32 matmuls of [128k,128m]^T @ [128k,512n] accumulating in PSUM,
  - scale/bias applied to the fp32 PSUM result on VectorE
    (weight stays exact fp16; only x is quantized to fp16).
"""

import sys

if "/opt/trn_rl_repo" not in sys.path:
    sys.path.insert(0, "/opt/trn_rl_repo")

import numpy as np

import concourse.bass as bass
import concourse.mybir as mybir
import concourse.tile as tile
from concourse import bacc

P = 128
N_CORES = 8
B, S, K, N = 4, 4096, 4096, 4096
M_TOTAL = B * S
M_SH = M_TOTAL // N_CORES  # 2048 rows per core
KO = K // P  # 32
MO = M_SH // P  # 16
N_TILE = 512
NO = N // N_TILE  # 8

F32 = mybir.dt.float32
F16 = mybir.dt.float16

_CACHED_NC = None


def _build_nc():
    nc = bacc.Bacc(None, target_bir_lowering=False, num_swdge_queues=4)

    x = nc.dram_tensor("x", (M_SH, K), F32, kind="ExternalInput")
    w = nc.dram_tensor("weight", (N, K), F16, kind="ExternalInput")
    scale = nc.dram_tensor("scale_weight", (N,), F32, kind="ExternalInput")
    bias = nc.dram_tensor("bias", (N,), F32, kind="ExternalInput")
    out = nc.dram_tensor("out", (M_SH, N), F32, kind="ExternalOutput")

    with tile.TileContext(nc) as tc:
        with (
            tc.tile_pool(name="dram", bufs=1, space="DRAM") as dram,
            tc.tile_pool(name="xT", bufs=1) as xtp,
            tc.tile_pool(name="wT", bufs=2) as wtp,
            tc.tile_pool(name="sbrep", bufs=2) as sbp,
            tc.tile_pool(name="psum", bufs=4, space="PSUM") as pp,
            tc.tile_pool(name="osb", bufs=3) as op,
        ):
            # ---- x: cast fp32 -> fp16 in DRAM, then XBAR-transpose into SBUF
            x16 = dram.tile((M_SH, K), F16)
            xT = xtp.tile((P, MO, KO, P), F16)  # xT[p, mo, ko, m] = x[mo*128+m, ko*128+p]
            for mo in range(MO):
                rows = slice(mo * P, (mo + 1) * P)
                nc.gpsimd.dma_start(out=x16[rows, :], in_=x[rows, :])
                nc.sync.dma_start_transpose(xT[:, mo], x16[rows, :])

            for no in range(NO):
                ncols = slice(no * N_TILE, (no + 1) * N_TILE)
                # wT[p, ko, n] = w[no*512+n, ko*128+p]
                wT = wtp.tile((P, KO, N_TILE), F16, tag="wT")
                nc.sync.dma_start_transpose(wT, w[ncols, :])

                # scale/bias slices replicated across all 128 partitions
                scale_rep = sbp.tile((P, N_TILE), F32, tag="scale")
                bias_rep = sbp.tile((P, N_TILE), F32, tag="bias")
                s_sl = scale[ncols]
                b_sl = bias[ncols]
                nc.gpsimd.dma_start(
                    out=scale_rep[:],
                    in_=bass.AP(tensor=s_sl.tensor, offset=s_sl.offset, ap=[[0, P], *s_sl.ap]),
                )
                nc.gpsimd.dma_start(
                    out=bias_rep[:],
                    in_=bass.AP(tensor=b_sl.tensor, offset=b_sl.offset, ap=[[0, P], *b_sl.ap]),
                )

                for mo in range(MO):
                    ps = pp.tile((P, N_TILE), F32, tag="ps")
                    for ko in range(KO):
                        nc.tensor.matmul(
                            ps[:],
                            lhsT=xT[:, mo, ko, :],
                            rhs=wT[:, ko, :],
                            start=(ko == 0),
                            stop=(ko == KO - 1),
                        )
                    ot = op.tile((P, N_TILE), F32, tag="ot")
                    nc.vector.tensor_mul(ot[:], ps[:], scale_rep[:])
                    nc.vector.tensor_add(ot[:], ot[:], bias_rep[:])
                    nc.sync.dma_start(out[mo * P:(mo + 1) * P, ncols], ot[:])

    nc.finalize()
    return nc


def _get_nc():
    global _CACHED_NC
    if _CACHED_NC is None:
        _CACHED_NC = _build_nc()
    return _CACHED_NC


def _run(inputs, trace=False, **spmd_kwargs):
    from concourse.bass_utils import run_bass_kernel_spmd

    x = np.asarray(inputs["x"], dtype=np.float32).reshape(M_TOTAL, K)
    w = np.ascontiguousarray(np.asarray(inputs["weight"], dtype=np.float16))
    scale = np.ascontiguousarray(np.asarray(inputs["scale_weight"], dtype=np.float32))
    bias = np.ascontiguousarray(np.asarray(inputs["bias"], dtype=np.float32))

    in_maps = []
    for c in range(N_CORES):
        in_maps.append(
            {
                "x": np.ascontiguousarray(x[c * M_SH:(c + 1) * M_SH]),
                "weight": w,
                "scale_weight": scale,
                "bias": bias,
            }
        )

    nc = _get_nc()
    res = run_bass_kernel_spmd(
        nc, in_maps, core_ids=list(range(N_CORES)), trace=trace, **spmd_kwargs
    )
    out = np.concatenate([res.results[c]["out"] for c in range(N_CORES)], axis=0)
    return out.reshape(B, S, N), res


def kernel(x, weight, scale_weight, bias):
    out, _ = _run({"x": x, "weight": weight, "scale_weight": scale_weight, "bias": bias})
    return out


# revision 4
# speedup vs baseline: 1.0513x; 1.0513x over previous
"""FP8ScaledLayer kernel for Trainium2 (8 NeuronCores, SPMD data-parallel).

Computes out = x @ (weight * scale[:, None]).T + bias with
  x: [4, 4096, 4096] fp32, weight: [4096, 4096] fp16,
  scale_weight: [4096] fp32, bias: [4096] fp32  ->  out [4, 4096, 4096] fp32.

Sharding: data-parallel over tokens (B*S = 16384 -> 2048 rows/core).
Weight is small (33.5 MB fp16) and replicated; x is large (268 MB) and
sharded, which keeps every core compute-bound instead of DMA-bound.

Per-core kernel:
  - cast x fp32 -> fp16 with a DRAM->DRAM SWDGE cast-DMA,
  - DMA-transpose (XBAR) x16 and weight into K-major SBUF tiles,
  - 16x8x32 matmuls of [128k,128m]^T @ [128k,512n] accumulating in PSUM,
  - scale/bias applied to the fp32 PSUM result on VectorE
    (weight stays exact fp16; only x is quantized to fp16).
"""

import sys

if "/opt/trn_rl_repo" not in sys.path:
    sys.path.insert(0, "/opt/trn_rl_repo")

import numpy as np

import concourse.bass as bass
import concourse.mybir as mybir
import concourse.tile as tile
from concourse import bacc

P = 128
N_CORES = 8
B, S, K, N = 4, 4096, 4096, 4096
M_TOTAL = B * S
M_SH = M_TOTAL // N_CORES  # 2048 rows per core
KO = K // P  # 32
MO = M_SH // P  # 16
N_TILE = 512
NO = N // N_TILE  # 8

F32 = mybir.dt.float32
F16 = mybir.dt.float16

_CACHED_NC = None


def _build_nc():
    nc = bacc.Bacc(None, target_bir_lowering=False, num_swdge_queues=4)

    x = nc.dram_tensor("x", (M_SH, K), F32, kind="ExternalInput")
    w = nc.dram_tensor("weight", (N, K), F16, kind="ExternalInput")
    scale = nc.dram_tensor("scale_weight", (N,), F32, kind="ExternalInput")
    bias = nc.dram_tensor("bias", (N,), F32, kind="ExternalInput")
    out = nc.dram_tensor("out", (M_SH, N), F32, kind="ExternalOutput")

    with tile.TileContext(nc) as tc:
        with (
            tc.tile_pool(name="dram", bufs=1, space="DRAM") as dram,
            tc.tile_pool(name="xT", bufs=1) as xtp,
            tc.tile_pool(name="wT", bufs=2) as wtp,
            tc.tile_pool(name="sbrep", bufs=2) as sbp,
            tc.tile_pool(name="psum", bufs=4, space="PSUM") as pp,
            tc.tile_pool(name="osb", bufs=3) as op,
        ):
            def load_rep(pool_tile, src_handle, ncols):
                sl = src_handle[ncols]
                nc.gpsimd.dma_start(
                    out=pool_tile[:],
                    in_=bass.AP(tensor=sl.tensor, offset=sl.offset, ap=[[0, P], *sl.ap]),
                )

            # ---- x: cast fp32 -> fp16 in DRAM, then XBAR-transpose into SBUF.
            # Emission order = per-queue execution order; the XBAR transpose
            # queue is the serial resource that gates startup, so order it:
            # cast(mo=0) split 4-way for fast start, then xT0, then wT0 split
            # 4-way (first matmul can start after xT0 + first wT0 chunk),
            # then the remaining casts/transposes which outpace PE consumption.
            x16 = dram.tile((M_SH, K), F16)
            xT = xtp.tile((P, MO, KO, P), F16)  # xT[p, mo, ko, m] = x[mo*128+m, ko*128+p]
            wts = []
            sreps = []
            for j in range(4):
                cols = slice(j * (K // 4), (j + 1) * (K // 4))
                nc.gpsimd.dma_start(out=x16[0:P, cols], in_=x[0:P, cols])
            nc.sync.dma_start_transpose(xT[:, 0], x16[0:P, :])

            wT0 = wtp.tile((P, KO, N_TILE), F16, tag="wT")
            for j in range(4):
                nc.sync.dma_start_transpose(
                    wT0[:, 8 * j:8 * (j + 1), :],
                    w[0:N_TILE, 1024 * j:1024 * (j + 1)],
                )
            wts.append(wT0)
            scale_rep0 = sbp.tile((P, N_TILE), F32, tag="scale")
            bias_rep0 = sbp.tile((P, N_TILE), F32, tag="bias")
            load_rep(scale_rep0, scale, slice(0, N_TILE))
            load_rep(bias_rep0, bias, slice(0, N_TILE))
            sreps.append((scale_rep0, bias_rep0))

            for mo in range(1, MO):
                rows = slice(mo * P, (mo + 1) * P)
                nc.gpsimd.dma_start(out=x16[rows, :], in_=x[rows, :])
                nc.sync.dma_start_transpose(xT[:, mo], x16[rows, :])

            for no in range(NO):
                ncols = slice(no * N_TILE, (no + 1) * N_TILE)
                if no == 0:
                    wT = wts[0]
                    scale_rep, bias_rep = sreps[0]
                else:
                    # wT[p, ko, n] = w[no*512+n, ko*128+p]
                    wT = wtp.tile((P, KO, N_TILE), F16, tag="wT")
                    nc.sync.dma_start_transpose(wT, w[ncols, :])
                    scale_rep = sbp.tile((P, N_TILE), F32, tag="scale")
                    bias_rep = sbp.tile((P, N_TILE), F32, tag="bias")
                    load_rep(scale_rep, scale, ncols)
                    load_rep(bias_rep, bias, ncols)

                for mo in range(MO):
                    ps = pp.tile((P, N_TILE), F32, tag="ps")
                    for ko in range(KO):
                        nc.tensor.matmul(
                            ps[:],
                            lhsT=xT[:, mo, ko, :],
                            rhs=wT[:, ko, :],
                            start=(ko == 0),
                            stop=(ko == KO - 1),
                        )
                    ot = op.tile((P, N_TILE), F32, tag="ot")
                    nc.vector.tensor_mul(ot[:], ps[:], scale_rep[:])
                    nc.vector.tensor_add(ot[:], ot[:], bias_rep[:])
                    # scalar-engine HWDGE queue: keeps output writes off the
                    # XBAR-transpose (sync) queue
                    nc.scalar.dma_start(out[mo * P:(mo + 1) * P, ncols], ot[:])

    nc.finalize()
    return nc


def _get_nc():
    global _CACHED_NC
    if _CACHED_NC is None:
        _CACHED_NC = _build_nc()
    return _CACHED_NC


def _run(inputs, trace=False, **spmd_kwargs):
    from concourse.bass_utils import run_bass_kernel_spmd

    x = np.asarray(inputs["x"], dtype=np.float32).reshape(M_TOTAL, K)
    w = np.ascontiguousarray(np.asarray(inputs["weight"], dtype=np.float16))
    scale = np.ascontiguousarray(np.asarray(inputs["scale_weight"], dtype=np.float32))
    bias = np.ascontiguousarray(np.asarray(inputs["bias"], dtype=np.float32))

    in_maps = []
    for c in range(N_CORES):
        in_maps.append(
            {
                "x": np.ascontiguousarray(x[c * M_SH:(c + 1) * M_SH]),
                "weight": w,
                "scale_weight": scale,
                "bias": bias,
            }
        )

    nc = _get_nc()
    res = run_bass_kernel_spmd(
        nc, in_maps, core_ids=list(range(N_CORES)), trace=trace, **spmd_kwargs
    )
    out = np.concatenate([res.results[c]["out"] for c in range(N_CORES)], axis=0)
    return out.reshape(B, S, N), res


def kernel(x, weight, scale_weight, bias):
    out, _ = _run({"x": x, "weight": weight, "scale_weight": scale_weight, "bias": bias})
    return out
